# revision 4
# baseline (speedup 1.0000x reference)
# Dissipation network Bass kernel for TRN2 (bf16 matmuls, PG-stacked PSUM).
#
# v2: merged per-PG {s|hole|x} matmuls for L2/L3 (one K=50,M=114 instruction
# computes both the s-gate preact and the next x preact per PG, with mirrored
# layouts per PG so DVE gate multiplies see equal operand partition bases),
# plus a combined-K (m1|x0) matmul that folds the W_clin gate matmul into the
# xs1 preact. Softplus = Exp (per-partition bias) then Ln(t + 1), pinned to
# the natural_log_exp_and_others ACT table.
import numpy as np
import ml_dtypes
import concourse.bass as bass
from concourse import bacc
import concourse.hw_specs as hw_specs
import concourse.bacc as bacc_mod
import concourse.mybir as mybir
import concourse.tile as tile

dt = mybir.dt
AF = mybir.ActivationFunctionType
ALU = mybir.AluOpType

_orig_get_tables = hw_specs.get_activation_tables


def _pinned_tables(arch):
    t = _orig_get_tables(arch)
    out = {}
    for name, fns in t.items():
        if name != "natural_log_exp_and_others":
            fns = fns - {AF.Exp, AF.Ln}
        out[name] = fns
    return out


bacc_mod.get_activation_tables = _pinned_tables

D, H = 16, 50
F = 1024            # free columns per PG block
STB = 2 * F         # rows per super-tile
NCHUNK = F // 512   # 512-col matmul chunks per F

W_SPECS = [
    ("W_xin", D, H), ("W_clinm", D, D),
    ("W_xl2", H, H), ("W_cl1m", H, D), ("W_cl1", D, H), ("W_cp1", H, H),
    ("W_xl3", H, H), ("W_cl2m", H, D), ("W_cl2", D, H), ("W_cp2", H, H),
    ("W_xlo", H, 1), ("W_cpom", H, H), ("W_clom", H, D),
    ("W_cpo", H, 1), ("W_clo", D, 1),
]
W_KM = {n: (k, m) for n, k, m in W_SPECS}
X0_WEIGHTS = {"W_xin", "W_clinm"}
W_OFF = {}
_off = 0
for _n, _k, _m in W_SPECS:
    W_OFF[_n] = _off
    _off += _m
W_OFF["W_xs1c"] = _off; _off += H      # combined clin|xl1, K=48
W_OFF["W_m2"] = _off; _off += 114      # merged cp1m|xp1 (mirrored per PG)
W_OFF["W_m3"] = _off; _off += 114      # merged cp2m|xp2
NW = _off

B_SPECS = ["b_xin", "b_clinm", "b_xl1", "b_xl2", "b_cl1m",
           "b_xl3", "b_cl2m", "b_xlo", "b_cpom", "b_clom",
           "b_m2_0", "b_m2_1", "b_m3_0", "b_m3_1"]
B_COL = {n: i for i, n in enumerate(B_SPECS)}
NB = len(B_SPECS)


def pack_weights(inputs):
    def wT(n):
        return np.asarray(inputs[n]).astype(np.float32).T.astype(ml_dtypes.bfloat16)

    wpack = np.zeros((128, NW), dtype=ml_dtypes.bfloat16)
    for n, k, m in W_SPECS:
        wb = wT(n)
        rb = 32 if n in X0_WEIGHTS else 0
        wpack[rb:rb + k, W_OFF[n]:W_OFF[n] + m] = wb
        wpack[64 + rb:64 + rb + k, W_OFF[n]:W_OFF[n] + m] = wb
    c = W_OFF["W_xs1c"]
    wpack[0:16, c:c + H] = wT("W_clin")
    wpack[32:48, c:c + H] = wT("W_xl1")
    wpack[64:80, c:c + H] = wT("W_clin")
    wpack[96:112, c:c + H] = wT("W_xl1")
    # merged {s|hole|x}: PG0 block (rows 0:50) -> out {s@0:50, x@64:114};
    # PG1 block (rows 64:114) -> out {x@0:50, s@64:114}
    for name, ws, wx, flip in (("W_m2", "W_cp1m", "W_xp1", False),
                               ("W_m3", "W_cp2m", "W_xp2", True)):
        c = W_OFF[name]
        sb16, xb16 = wT(ws), wT(wx)
        lo_s, hi_s = (False, True) if flip else (True, False)
        # non-flip: rows 0:50 hold PG0 block {s@0:50, x@64:114};
        # flip (L3): rows 0:50 hold PG1 block {x@0:50, s@64:114}
        if flip:
            wpack[0:50, c:c + 50] = xb16
            wpack[0:50, c + 64:c + 114] = sb16
            wpack[64:114, c:c + 50] = sb16
            wpack[64:114, c + 64:c + 114] = xb16
        else:
            wpack[0:50, c:c + 50] = sb16
            wpack[0:50, c + 64:c + 114] = xb16
            wpack[64:114, c:c + 50] = xb16
            wpack[64:114, c + 64:c + 114] = sb16

    bpack = np.zeros((128, NB), dtype=np.float32)

    def bvec(n):
        return np.asarray(inputs[n]).astype(np.float32)

    for n in ["b_xin", "b_clinm", "b_xl1", "b_xl2", "b_cl1m",
              "b_xl3", "b_cl2m", "b_xlo", "b_cpom", "b_clom"]:
        b = bvec(n)
        cix = B_COL[n]
        bpack[0:len(b), cix] = b
        bpack[64:64 + len(b), cix] = b
    for name, bs, bx in (("b_m2", "b_cp1m", "b_xp1"), ("b_m3", "b_cp2m", "b_xp2")):
        s, x = bvec(bs), bvec(bx)
        bpack[0:50, B_COL[name + "_0"]] = s
        bpack[64:114, B_COL[name + "_0"]] = x
        bpack[0:50, B_COL[name + "_1"]] = x
        bpack[64:114, B_COL[name + "_1"]] = s
    ident = np.eye(128, dtype=np.float32)
    return wpack, bpack, ident


def build_program(n_rows):
    assert n_rows % STB == 0
    nst = n_rows // STB
    nc = bacc.Bacc("TRN2", target_bir_lowering=False, debug=False,
                   enable_asserts=False)
    inp_d = nc.dram_tensor("input", [n_rows, 32], dt.float32, kind="ExternalInput")
    w_d = nc.dram_tensor("wpack", [128, NW], dt.bfloat16, kind="ExternalInput")
    b_d = nc.dram_tensor("bpack", [128, NB], dt.float32, kind="ExternalInput")
    c_d = nc.dram_tensor("ident", [128, 128], dt.float32, kind="ExternalInput")
    out_d = nc.dram_tensor("out", [n_rows, 1], dt.float32, kind="ExternalOutput")

    with tile.TileContext(nc) as tc:
        with tc.tile_pool(name="const", bufs=1) as cpool, \
             tc.tile_pool(name="inp", bufs=4) as inpool, \
             tc.tile_pool(name="x0p", bufs=4) as x0pool, \
             tc.tile_pool(name="mh", bufs=6) as mhpool, \
             tc.tile_pool(name="g", bufs=4) as gpool, \
             tc.tile_pool(name="stg", bufs=8) as stgpool, \
             tc.tile_pool(name="axs", bufs=8) as xspool, \
             tc.tile_pool(name="ax", bufs=4) as xpool, \
             tc.tile_pool(name="uu", bufs=6) as upool, \
             tc.tile_pool(name="aout", bufs=3) as outpool, \
             tc.tile_pool(name="ps", bufs=4, space="PSUM") as ps:

            wt = cpool.tile([128, NW], dt.bfloat16)
            nc.sync.dma_start(out=wt[:], in_=w_d.ap())
            bt = cpool.tile([128, NB], dt.float32)
            nc.sync.dma_start(out=bt[:], in_=b_d.ap())
            ct = cpool.tile([128, 128], dt.float32)
            nc.sync.dma_start(out=ct[:], in_=c_d.ap())

            def mm_pg(psum_t, wname, wm, rhs_t, rrow, k, orow,
                      start, stop, c):
                off = W_OFF[wname]
                cs = slice(512 * c, 512 * (c + 1))
                nc.tensor.matmul(psum_t[orow:orow + wm, cs],
                                 wt[rrow:rrow + k, off:off + wm],
                                 rhs_t[rrow:rrow + k, cs],
                                 start=start, stop=stop,
                                 tile_position=(rrow, orow))

            def softplus_rows(psum_t, r1, bias_name, out_t):
                stg = stgpool.tile([r1, F], dt.float32, tag="stg")
                nc.scalar.activation(stg[0:r1, :], psum_t[0:r1, :], AF.Exp,
                                     bias=bt[0:r1, B_COL[bias_name]:B_COL[bias_name] + 1])
                nc.scalar.activation(out_t[0:r1, :], stg[0:r1, :], AF.Ln, bias=1.0)

            def body(st):
                r0 = st * STB
                in_t = inpool.tile([128, 8 * 112], dt.float32, tag="int")
                for pg in range(2):
                    rb = r0 + pg * F
                    src_x = inp_d.ap()[rb:rb + F, 0:16].rearrange("(a p) f -> p a f", p=128)
                    src_s = inp_d.ap()[rb:rb + F, 16:32].rearrange("(a p) f -> p a f", p=128)
                    r3 = in_t[:].rearrange("p (a q) -> p a q", q=112)
                    nc.sync.dma_start(out=r3[:, :, 64 * pg + 32:64 * pg + 48], in_=src_x)
                    nc.sync.dma_start(out=r3[:, :, 64 * pg:64 * pg + 16], in_=src_s)
                    # dup x0s into the 16:32 hole so the combined xs1 matmul
                    # (zero weights there) never multiplies stale SBUF bits
                    nc.sync.dma_start(out=r3[:, :, 64 * pg + 16:64 * pg + 32], in_=src_s)
                pT = ps.tile([112, F], dt.float32, tag="ps")
                for a in range(8):
                    nc.tensor.transpose(pT[0:112, 128 * a:128 * (a + 1)],
                                        in_t[:, 112 * a:112 * a + 112], ct[:])
                x0b = x0pool.tile([112, F], dt.bfloat16, tag="x0b")
                nc.vector.tensor_copy(x0b[0:112, :], pT[0:112, :])
                x0s = x0pool.tile([80, F], dt.float32, tag="x0s")
                nc.vector.tensor_copy(x0s[0:80, :], pT[0:80, :])

                # ---- L1 ----
                p_x1 = ps.tile([114, F], dt.float32, tag="ps")
                for c in range(NCHUNK):
                    mm_pg(p_x1, "W_xin", 50, x0b, 32, 16, 0, True, True, c)
                for c in range(NCHUNK):
                    mm_pg(p_x1, "W_xin", 50, x0b, 96, 16, 64, True, True, c)
                a_x1 = xpool.tile([114, F], dt.bfloat16, tag="ax")
                softplus_rows(p_x1, 114, "b_xin", a_x1)

                p_dm = ps.tile([80, F], dt.float32, tag="ps")
                for c in range(NCHUNK):
                    mm_pg(p_dm, "W_clinm", 16, x0b, 32, 16, 0, True, True, c)
                for c in range(NCHUNK):
                    mm_pg(p_dm, "W_clinm", 16, x0b, 96, 16, 64, True, True, c)
                # m1 -> x0b rows 0:16 / 64:80 (bf16), per PG
                for base in (0, 64):
                    nc.vector.scalar_tensor_tensor(
                        x0b[base:base + 16, :], p_dm[base:base + 16, :],
                        bt[base:base + 16, B_COL["b_clinm"]:B_COL["b_clinm"] + 1],
                        x0s[base:base + 16, :], op0=ALU.add, op1=ALU.mult)
                # xs1 = xl1*x0 + clin*m1 via combined K=48 rhs x0b[0:48]/[64:112]
                p_xs1 = ps.tile([114, F], dt.float32, tag="ps")
                for c in range(NCHUNK):
                    mm_pg(p_xs1, "W_xs1c", 50, x0b, 0, 48, 0, True, True, c)
                for c in range(NCHUNK):
                    mm_pg(p_xs1, "W_xs1c", 50, x0b, 64, 48, 64, True, True, c)
                a_xs1 = xspool.tile([114, F], dt.bfloat16, tag="axs")
                softplus_rows(p_xs1, 114, "b_xl1", a_xs1)
                return dict(r0=r0, x0s=x0s, a_xs1=a_xs1, a_x1=a_x1)

            def layer(a_prev, pg0_r, pg1_r, a_xs, x0s, wm, bm, wdh, bdh,
                      wcl, wxl, wcp, bxs):
                # a_prev x-slots: PG0 at rows pg0_r:pg0_r+50 (tile t0), PG1 at
                # pg1_r:pg1_r+50 (tile t1). For stacked input t0 is t1.
                t0, t1 = a_prev
                A0 = ps.tile([114, F], dt.float32, tag="ps")
                A1 = ps.tile([114, F], dt.float32, tag="ps")
                p_dh = ps.tile([80, F], dt.float32, tag="ps")
                p_xs = ps.tile([114, F], dt.float32, tag="ps")
                for c in range(NCHUNK):
                    mm_pg(A0, wm, 114, t0, pg0_r, 50, 0, True, True, c)
                for c in range(NCHUNK):
                    mm_pg(A1, wm, 114, t1, pg1_r, 50, 0, True, True, c)
                for c in range(NCHUNK):
                    mm_pg(p_dh, wdh, 16, t0, pg0_r, 50, 0, True, True, c)
                for c in range(NCHUNK):
                    mm_pg(p_dh, wdh, 16, t1, pg1_r, 50, 64, True, True, c)
                for c in range(NCHUNK):
                    mm_pg(p_xs, wxl, 50, t0, pg0_r, 50, 0, True, False, c)
                for c in range(NCHUNK):
                    mm_pg(p_xs, wxl, 50, t1, pg1_r, 50, 64, True, False, c)
                U0 = upool.tile([114, F], dt.bfloat16, tag="uu")
                U1 = upool.tile([114, F], dt.bfloat16, tag="uu")
                softplus_rows(A0, 114, bm + "_0", U0)
                softplus_rows(A1, 114, bm + "_1", U1)
                h = mhpool.tile([80, F], dt.bfloat16, tag="mh")
                nc.vector.scalar_tensor_tensor(
                    h[0:80, :], p_dh[0:80, :],
                    bt[0:80, B_COL[bdh]:B_COL[bdh] + 1],
                    x0s[0:80, :], op0=ALU.add, op1=ALU.mult)
                g = gpool.tile([114, F], dt.bfloat16, tag="g")
                nc.vector.tensor_tensor(g[0:50, :], a_xs[0:50, :], U0[0:50, :],
                                        op=ALU.mult)
                nc.vector.tensor_tensor(g[64:114, :], a_xs[64:114, :],
                                        U1[64:114, :], op=ALU.mult)
                for c in range(NCHUNK):
                    mm_pg(p_xs, wcl, 50, h, 0, 16, 0, False, False, c)
                for c in range(NCHUNK):
                    mm_pg(p_xs, wcl, 50, h, 64, 16, 64, False, False, c)
                for c in range(NCHUNK):
                    mm_pg(p_xs, wcp, 50, g, 0, 50, 0, False, c == NCHUNK - 1, c)
                for c in range(NCHUNK):
                    mm_pg(p_xs, wcp, 50, g, 64, 50, 64, False, c == NCHUNK - 1, c)
                a_xs_n = xspool.tile([114, F], dt.bfloat16, tag="axs")
                softplus_rows(p_xs, 114, bxs, a_xs_n)
                return U0, U1, a_xs_n

            def back(cin):
                r0, x0s, a_xs1, a_x1 = cin["r0"], cin["x0s"], cin["a_xs1"], cin["a_x1"]
                U20, U21, a_xs2 = layer((a_x1, a_x1), 0, 64, a_xs1, x0s,
                                        "W_m2", "b_m2", "W_cl1m", "b_cl1m",
                                        "W_cl1", "W_xl2", "W_cp1", "b_xl2")
                # L3: x2 at U20[64:114] (PG0) and U21[0:50] (PG1)
                U30, U31, a_xs3 = layer((U20, U21), 64, 0, a_xs2, x0s,
                                        "W_m3", "b_m3", "W_cl2m", "b_cl2m",
                                        "W_cl2", "W_xl3", "W_cp2", "b_xl3")
                # ---- L4 ----
                p_s3 = ps.tile([114, F], dt.float32, tag="ps")
                p_dh3 = ps.tile([80, F], dt.float32, tag="ps")
                p_out = ps.tile([65, F], dt.float32, tag="ps")
                for c in range(NCHUNK):
                    mm_pg(p_s3, "W_cpom", 50, U30, 64, 50, 0, True, True, c)
                for c in range(NCHUNK):
                    mm_pg(p_s3, "W_cpom", 50, U31, 0, 50, 64, True, True, c)
                for c in range(NCHUNK):
                    mm_pg(p_dh3, "W_clom", 16, U30, 64, 50, 0, True, True, c)
                for c in range(NCHUNK):
                    mm_pg(p_dh3, "W_clom", 16, U31, 0, 50, 64, True, True, c)
                for c in range(NCHUNK):
                    mm_pg(p_out, "W_xlo", 1, U30, 64, 50, 0, True, False, c)
                for c in range(NCHUNK):
                    mm_pg(p_out, "W_xlo", 1, U31, 0, 50, 64, True, False, c)
                a_s3 = xspool.tile([114, F], dt.bfloat16, tag="axs")
                softplus_rows(p_s3, 114, "b_cpom", a_s3)
                h3 = mhpool.tile([80, F], dt.bfloat16, tag="mh")
                nc.vector.scalar_tensor_tensor(
                    h3[0:80, :], p_dh3[0:80, :],
                    bt[0:80, B_COL["b_clom"]:B_COL["b_clom"] + 1],
                    x0s[0:80, :], op0=ALU.add, op1=ALU.mult)
                g3 = gpool.tile([114, F], dt.bfloat16, tag="g")
                nc.vector.tensor_tensor(g3[0:114, :], a_xs3[0:114, :],
                                        a_s3[0:114, :], op=ALU.mult)
                for c in range(NCHUNK):
                    mm_pg(p_out, "W_clo", 1, h3, 0, 16, 0, False, False, c)
                for c in range(NCHUNK):
                    mm_pg(p_out, "W_clo", 1, h3, 64, 16, 64, False, False, c)
                for c in range(NCHUNK):
                    mm_pg(p_out, "W_cpo", 1, g3, 0, 50, 0, False, c == NCHUNK - 1, c)
                for c in range(NCHUNK):
                    mm_pg(p_out, "W_cpo", 1, g3, 64, 50, 64, False, c == NCHUNK - 1, c)
                return dict(r0=r0, p_out=p_out)

            def tail(c2):
                r0, p_out = c2["r0"], c2["p_out"]
                a_out = outpool.tile([65, F], dt.float32, tag="aout")
                softplus_rows(p_out, 65, "b_xlo", a_out)
                nc.sync.dma_start(out=out_d.ap()[r0:r0 + F, 0:1], in_=a_out[0:1, :])
                nc.sync.dma_start(out=out_d.ap()[r0 + F:r0 + STB, 0:1], in_=a_out[64:65, :])

            pending = None
            pending2 = None
            for st in range(nst):
                c = body(st)
                if pending is not None:
                    c2 = back(pending)
                    if pending2 is not None:
                        tail(pending2)
                    pending2 = c2
                pending = c
            pending2b = back(pending)
            tail(pending2)
            tail(pending2b)

    nc.finalize()
    return nc


# ---------------------------------------------------------------------------
N_CORES = 8
_program_cache = {}


def _get_program(core_rows):
    if core_rows not in _program_cache:
        _program_cache[core_rows] = build_program(core_rows)
    return _program_cache[core_rows]


def kernel(**inputs):
    from concourse.bass_utils import run_bass_kernel_spmd
    x = np.ascontiguousarray(np.asarray(inputs["input"], dtype=np.float32))
    B = x.shape[0]
    assert x.shape[1] == 2 * D
    core_rows = B // N_CORES
    assert core_rows * N_CORES == B and core_rows % STB == 0, (B,)
    wpack, bpack, ident = pack_weights(inputs)
    nc = _get_program(core_rows)
    in_maps = [{
        "input": x[i * core_rows:(i + 1) * core_rows],
        "wpack": wpack, "bpack": bpack, "ident": ident,
    } for i in range(N_CORES)]
    res = run_bass_kernel_spmd(nc, in_maps, list(range(N_CORES)))
    return np.concatenate([res.results[i]["out"] for i in range(N_CORES)], axis=0)


# revision 6
# speedup vs baseline: 1.1048x; 1.1048x over previous
# Dissipation network Bass kernel for TRN2 (bf16 matmuls, PG-stacked PSUM).
#
# v2: merged per-PG {s|hole|x} matmuls for L2/L3 (one K=50,M=114 instruction
# computes both the s-gate preact and the next x preact per PG, with mirrored
# layouts per PG so DVE gate multiplies see equal operand partition bases),
# plus a combined-K (m1|x0) matmul that folds the W_clin gate matmul into the
# xs1 preact. Softplus = Exp (per-partition bias) then Ln(t + 1), pinned to
# the natural_log_exp_and_others ACT table.
import numpy as np
import ml_dtypes
import concourse.bass as bass
from concourse import bacc
import concourse.hw_specs as hw_specs
import concourse.bacc as bacc_mod
import concourse.mybir as mybir
import concourse.tile as tile

dt = mybir.dt
AF = mybir.ActivationFunctionType
ALU = mybir.AluOpType

_orig_get_tables = hw_specs.get_activation_tables


def _pinned_tables(arch):
    t = _orig_get_tables(arch)
    out = {}
    for name, fns in t.items():
        if name != "natural_log_exp_and_others":
            fns = fns - {AF.Exp, AF.Ln}
        out[name] = fns
    return out


bacc_mod.get_activation_tables = _pinned_tables

D, H = 16, 50
F = 1024            # free columns per PG block
STB = 2 * F         # rows per super-tile
NCHUNK = F // 512   # 512-col matmul chunks per F

W_SPECS = [
    ("W_xin", D, H), ("W_clinm", D, D),
    ("W_xl2", H, H), ("W_cl1m", H, D), ("W_cl1", D, H), ("W_cp1", H, H),
    ("W_xl3", H, H), ("W_cl2m", H, D), ("W_cl2", D, H), ("W_cp2", H, H),
    ("W_xlo", H, 1), ("W_cpom", H, H), ("W_clom", H, D),
    ("W_cpo", H, 1), ("W_clo", D, 1),
]
W_KM = {n: (k, m) for n, k, m in W_SPECS}
X0_WEIGHTS = {"W_xin", "W_clinm"}
W_OFF = {}
_off = 0
for _n, _k, _m in W_SPECS:
    W_OFF[_n] = _off
    _off += _m
W_OFF["W_xs1c"] = _off; _off += H      # combined clin|xl1, K=48
W_OFF["W_m2"] = _off; _off += 114      # merged cp1m|xp1 (mirrored per PG)
W_OFF["W_m3"] = _off; _off += 114      # merged cp2m|xp2
NW = _off

B_SPECS = ["b_xin", "b_clinm", "b_xl1", "b_xl2", "b_cl1m",
           "b_xl3", "b_cl2m", "b_xlo", "b_cpom", "b_clom",
           "b_m2_0", "b_m2_1", "b_m3_0", "b_m3_1"]
B_COL = {n: i for i, n in enumerate(B_SPECS)}
NB = len(B_SPECS)


def pack_weights(inputs):
    def wT(n):
        return np.asarray(inputs[n]).astype(np.float32).T.astype(ml_dtypes.bfloat16)

    wpack = np.zeros((128, NW), dtype=ml_dtypes.bfloat16)
    for n, k, m in W_SPECS:
        wb = wT(n)
        rb = 32 if n in X0_WEIGHTS else 0
        wpack[rb:rb + k, W_OFF[n]:W_OFF[n] + m] = wb
        wpack[64 + rb:64 + rb + k, W_OFF[n]:W_OFF[n] + m] = wb
    c = W_OFF["W_xs1c"]
    wpack[0:16, c:c + H] = wT("W_clin")
    wpack[32:48, c:c + H] = wT("W_xl1")
    wpack[64:80, c:c + H] = wT("W_clin")
    wpack[96:112, c:c + H] = wT("W_xl1")
    # merged {s|hole|x}: PG0 block (rows 0:50) -> out {s@0:50, x@64:114};
    # PG1 block (rows 64:114) -> out {x@0:50, s@64:114}
    for name, ws, wx, flip in (("W_m2", "W_cp1m", "W_xp1", False),
                               ("W_m3", "W_cp2m", "W_xp2", True)):
        c = W_OFF[name]
        sb16, xb16 = wT(ws), wT(wx)
        lo_s, hi_s = (False, True) if flip else (True, False)
        # non-flip: rows 0:50 hold PG0 block {s@0:50, x@64:114};
        # flip (L3): rows 0:50 hold PG1 block {x@0:50, s@64:114}
        if flip:
            wpack[0:50, c:c + 50] = xb16
            wpack[0:50, c + 64:c + 114] = sb16
            wpack[64:114, c:c + 50] = sb16
            wpack[64:114, c + 64:c + 114] = xb16
        else:
            wpack[0:50, c:c + 50] = sb16
            wpack[0:50, c + 64:c + 114] = xb16
            wpack[64:114, c:c + 50] = xb16
            wpack[64:114, c + 64:c + 114] = sb16

    bpack = np.zeros((128, NB), dtype=np.float32)

    def bvec(n):
        return np.asarray(inputs[n]).astype(np.float32)

    for n in ["b_xin", "b_clinm", "b_xl1", "b_xl2", "b_cl1m",
              "b_xl3", "b_cl2m", "b_xlo", "b_cpom", "b_clom"]:
        b = bvec(n)
        cix = B_COL[n]
        bpack[0:len(b), cix] = b
        bpack[64:64 + len(b), cix] = b
    for name, bs, bx in (("b_m2", "b_cp1m", "b_xp1"), ("b_m3", "b_cp2m", "b_xp2")):
        s, x = bvec(bs), bvec(bx)
        bpack[0:50, B_COL[name + "_0"]] = s
        bpack[64:114, B_COL[name + "_0"]] = x
        bpack[0:50, B_COL[name + "_1"]] = x
        bpack[64:114, B_COL[name + "_1"]] = s
    ident = np.eye(128, dtype=np.float32)
    return wpack, bpack, ident


def build_program(n_rows):
    assert n_rows % STB == 0
    nst = n_rows // STB
    nc = bacc.Bacc("TRN2", target_bir_lowering=False, debug=False,
                   enable_asserts=False)
    inp_d = nc.dram_tensor("input", [n_rows, 32], dt.float32, kind="ExternalInput")
    w_d = nc.dram_tensor("wpack", [128, NW], dt.bfloat16, kind="ExternalInput")
    b_d = nc.dram_tensor("bpack", [128, NB], dt.float32, kind="ExternalInput")
    c_d = nc.dram_tensor("ident", [128, 128], dt.float32, kind="ExternalInput")
    out_d = nc.dram_tensor("out", [n_rows, 1], dt.float32, kind="ExternalOutput")

    with tile.TileContext(nc) as tc:
        with tc.tile_pool(name="const", bufs=1) as cpool, \
             tc.tile_pool(name="inp", bufs=4) as inpool, \
             tc.tile_pool(name="x0p", bufs=4) as x0pool, \
             tc.tile_pool(name="mh", bufs=6) as mhpool, \
             tc.tile_pool(name="g", bufs=4) as gpool, \
             tc.tile_pool(name="stg", bufs=8) as stgpool, \
             tc.tile_pool(name="axs", bufs=8) as xspool, \
             tc.tile_pool(name="ax", bufs=4) as xpool, \
             tc.tile_pool(name="uu", bufs=6) as upool, \
             tc.tile_pool(name="aout", bufs=3) as outpool, \
             tc.tile_pool(name="ps", bufs=3, space="PSUM") as ps, \
             tc.tile_pool(name="po", bufs=1, space="PSUM") as po:

            wt = cpool.tile([128, NW], dt.bfloat16)
            nc.sync.dma_start(out=wt[:], in_=w_d.ap())
            bt = cpool.tile([128, NB], dt.float32)
            nc.sync.dma_start(out=bt[:], in_=b_d.ap())
            ct = cpool.tile([128, 128], dt.float32)
            nc.sync.dma_start(out=ct[:], in_=c_d.ap())

            def mm_pg(psum_t, wname, wm, rhs_t, rrow, k, orow,
                      start, stop, c):
                off = W_OFF[wname]
                cs = slice(512 * c, 512 * (c + 1))
                nc.tensor.matmul(psum_t[orow:orow + wm, cs],
                                 wt[rrow:rrow + k, off:off + wm],
                                 rhs_t[rrow:rrow + k, cs],
                                 start=start, stop=stop,
                                 tile_position=(rrow, orow))

            def softplus_rows(psum_t, r1, bias_name, out_t):
                stg = stgpool.tile([r1, F], dt.float32, tag="stg")
                nc.scalar.activation(stg[0:r1, :], psum_t[0:r1, :], AF.Exp,
                                     bias=bt[0:r1, B_COL[bias_name]:B_COL[bias_name] + 1])
                nc.scalar.activation(out_t[0:r1, :], stg[0:r1, :], AF.Ln, bias=1.0)

            def body(st):
                r0 = st * STB
                in_t = inpool.tile([128, 8 * 112], dt.float32, tag="int")
                for pg in range(2):
                    rb = r0 + pg * F
                    src_x = inp_d.ap()[rb:rb + F, 0:16].rearrange("(a p) f -> p a f", p=128)
                    src_s = inp_d.ap()[rb:rb + F, 16:32].rearrange("(a p) f -> p a f", p=128)
                    r3 = in_t[:].rearrange("p (a q) -> p a q", q=112)
                    nc.sync.dma_start(out=r3[:, :, 64 * pg + 32:64 * pg + 48], in_=src_x)
                    nc.sync.dma_start(out=r3[:, :, 64 * pg:64 * pg + 16], in_=src_s)
                    # dup x0s into the 16:32 hole so the combined xs1 matmul
                    # (zero weights there) never multiplies stale SBUF bits
                    nc.sync.dma_start(out=r3[:, :, 64 * pg + 16:64 * pg + 32], in_=src_s)
                pT = ps.tile([112, F], dt.float32, tag="ps")
                for a in range(8):
                    nc.tensor.transpose(pT[0:112, 128 * a:128 * (a + 1)],
                                        in_t[:, 112 * a:112 * a + 112], ct[:])
                x0b = x0pool.tile([112, F], dt.bfloat16, tag="x0b")
                nc.vector.tensor_copy(x0b[0:112, :], pT[0:112, :])
                x0s = x0pool.tile([80, F], dt.float32, tag="x0s")
                nc.vector.tensor_copy(x0s[0:80, :], pT[0:80, :])

                # ---- L1 ----
                p_x1 = ps.tile([114, F], dt.float32, tag="ps")
                for c in range(NCHUNK):
                    mm_pg(p_x1, "W_xin", 50, x0b, 32, 16, 0, True, True, c)
                for c in range(NCHUNK):
                    mm_pg(p_x1, "W_xin", 50, x0b, 96, 16, 64, True, True, c)
                a_x1 = xpool.tile([114, F], dt.bfloat16, tag="ax")
                softplus_rows(p_x1, 114, "b_xin", a_x1)

                p_dm = ps.tile([80, F], dt.float32, tag="ps")
                for c in range(NCHUNK):
                    mm_pg(p_dm, "W_clinm", 16, x0b, 32, 16, 0, True, True, c)
                for c in range(NCHUNK):
                    mm_pg(p_dm, "W_clinm", 16, x0b, 96, 16, 64, True, True, c)
                # m1 -> x0b rows 0:16 / 64:80 (bf16), per PG
                for base in (0, 64):
                    nc.vector.scalar_tensor_tensor(
                        x0b[base:base + 16, :], p_dm[base:base + 16, :],
                        bt[base:base + 16, B_COL["b_clinm"]:B_COL["b_clinm"] + 1],
                        x0s[base:base + 16, :], op0=ALU.add, op1=ALU.mult)
                # xs1 = xl1*x0 + clin*m1 via combined K=48 rhs x0b[0:48]/[64:112]
                p_xs1 = ps.tile([114, F], dt.float32, tag="ps")
                for c in range(NCHUNK):
                    mm_pg(p_xs1, "W_xs1c", 50, x0b, 0, 48, 0, True, True, c)
                for c in range(NCHUNK):
                    mm_pg(p_xs1, "W_xs1c", 50, x0b, 64, 48, 64, True, True, c)
                a_xs1 = xspool.tile([114, F], dt.bfloat16, tag="axs")
                softplus_rows(p_xs1, 114, "b_xl1", a_xs1)
                return dict(r0=r0, x0s=x0s, a_xs1=a_xs1, a_x1=a_x1)

            def layer(a_prev, pg0_r, pg1_r, a_xs, x0s, wm, bm, wdh, bdh,
                      wcl, wxl, wcp, bxs):
                # a_prev x-slots: PG0 at rows pg0_r:pg0_r+50 (tile t0), PG1 at
                # pg1_r:pg1_r+50 (tile t1). For stacked input t0 is t1.
                t0, t1 = a_prev
                A0 = ps.tile([114, F], dt.float32, tag="ps")
                A1 = ps.tile([114, F], dt.float32, tag="ps")
                p_dh = ps.tile([80, F], dt.float32, tag="ps")
                p_xs = ps.tile([114, F], dt.float32, tag="ps")
                for c in range(NCHUNK):
                    mm_pg(A0, wm, 114, t0, pg0_r, 50, 0, True, True, c)
                for c in range(NCHUNK):
                    mm_pg(A1, wm, 114, t1, pg1_r, 50, 0, True, True, c)
                for c in range(NCHUNK):
                    mm_pg(p_dh, wdh, 16, t0, pg0_r, 50, 0, True, True, c)
                for c in range(NCHUNK):
                    mm_pg(p_dh, wdh, 16, t1, pg1_r, 50, 64, True, True, c)
                for c in range(NCHUNK):
                    mm_pg(p_xs, wxl, 50, t0, pg0_r, 50, 0, True, False, c)
                for c in range(NCHUNK):
                    mm_pg(p_xs, wxl, 50, t1, pg1_r, 50, 64, True, False, c)
                stg2 = stgpool.tile([114, 2 * F], dt.float32, tag="stg")
                nc.scalar.activation(stg2[0:114, 0:F], A0[0:114, :], AF.Exp,
                                     bias=bt[0:114, B_COL[bm + "_0"]:B_COL[bm + "_0"] + 1])
                nc.scalar.activation(stg2[0:114, F:2 * F], A1[0:114, :], AF.Exp,
                                     bias=bt[0:114, B_COL[bm + "_1"]:B_COL[bm + "_1"] + 1])
                U2 = upool.tile([114, 2 * F], dt.bfloat16, tag="uu")
                nc.scalar.activation(U2[0:114, :], stg2[0:114, :], AF.Ln, bias=1.0)
                U0 = U2[:, 0:F]
                U1 = U2[:, F:2 * F]
                h = mhpool.tile([80, F], dt.bfloat16, tag="mh")
                nc.vector.scalar_tensor_tensor(
                    h[0:80, :], p_dh[0:80, :],
                    bt[0:80, B_COL[bdh]:B_COL[bdh] + 1],
                    x0s[0:80, :], op0=ALU.add, op1=ALU.mult)
                g = gpool.tile([114, F], dt.bfloat16, tag="g")
                nc.vector.tensor_tensor(g[0:50, :], a_xs[0:50, :], U0[0:50, :],
                                        op=ALU.mult)
                nc.vector.tensor_tensor(g[64:114, :], a_xs[64:114, :],
                                        U1[64:114, :], op=ALU.mult)
                for c in range(NCHUNK):
                    mm_pg(p_xs, wcl, 50, h, 0, 16, 0, False, False, c)
                for c in range(NCHUNK):
                    mm_pg(p_xs, wcl, 50, h, 64, 16, 64, False, False, c)
                for c in range(NCHUNK):
                    mm_pg(p_xs, wcp, 50, g, 0, 50, 0, False, c == NCHUNK - 1, c)
                for c in range(NCHUNK):
                    mm_pg(p_xs, wcp, 50, g, 64, 50, 64, False, c == NCHUNK - 1, c)
                a_xs_n = xspool.tile([114, F], dt.bfloat16, tag="axs")
                softplus_rows(p_xs, 114, bxs, a_xs_n)
                return U0, U1, a_xs_n

            def back(cin):
                r0, x0s, a_xs1, a_x1 = cin["r0"], cin["x0s"], cin["a_xs1"], cin["a_x1"]
                U20, U21, a_xs2 = layer((a_x1, a_x1), 0, 64, a_xs1, x0s,
                                        "W_m2", "b_m2", "W_cl1m", "b_cl1m",
                                        "W_cl1", "W_xl2", "W_cp1", "b_xl2")
                # L3: x2 at U20[64:114] (PG0) and U21[0:50] (PG1)
                U30, U31, a_xs3 = layer((U20, U21), 64, 0, a_xs2, x0s,
                                        "W_m3", "b_m3", "W_cl2m", "b_cl2m",
                                        "W_cl2", "W_xl3", "W_cp2", "b_xl3")
                # ---- L4 ----
                p_s3 = ps.tile([114, F], dt.float32, tag="ps")
                p_dh3 = ps.tile([80, F], dt.float32, tag="ps")
                p_out = po.tile([65, F], dt.float32, tag="po")
                for c in range(NCHUNK):
                    mm_pg(p_s3, "W_cpom", 50, U30, 64, 50, 0, True, True, c)
                for c in range(NCHUNK):
                    mm_pg(p_s3, "W_cpom", 50, U31, 0, 50, 64, True, True, c)
                for c in range(NCHUNK):
                    mm_pg(p_dh3, "W_clom", 16, U30, 64, 50, 0, True, True, c)
                for c in range(NCHUNK):
                    mm_pg(p_dh3, "W_clom", 16, U31, 0, 50, 64, True, True, c)
                for c in range(NCHUNK):
                    mm_pg(p_out, "W_xlo", 1, U30, 64, 50, 0, True, False, c)
                for c in range(NCHUNK):
                    mm_pg(p_out, "W_xlo", 1, U31, 0, 50, 64, True, False, c)
                a_s3 = xspool.tile([114, F], dt.bfloat16, tag="axs")
                softplus_rows(p_s3, 114, "b_cpom", a_s3)
                h3 = mhpool.tile([80, F], dt.bfloat16, tag="mh")
                nc.vector.scalar_tensor_tensor(
                    h3[0:80, :], p_dh3[0:80, :],
                    bt[0:80, B_COL["b_clom"]:B_COL["b_clom"] + 1],
                    x0s[0:80, :], op0=ALU.add, op1=ALU.mult)
                g3 = gpool.tile([114, F], dt.bfloat16, tag="g")
                nc.vector.tensor_tensor(g3[0:114, :], a_xs3[0:114, :],
                                        a_s3[0:114, :], op=ALU.mult)
                for c in range(NCHUNK):
                    mm_pg(p_out, "W_clo", 1, h3, 0, 16, 0, False, False, c)
                for c in range(NCHUNK):
                    mm_pg(p_out, "W_clo", 1, h3, 64, 16, 64, False, False, c)
                for c in range(NCHUNK):
                    mm_pg(p_out, "W_cpo", 1, g3, 0, 50, 0, False, c == NCHUNK - 1, c)
                for c in range(NCHUNK):
                    mm_pg(p_out, "W_cpo", 1, g3, 64, 50, 64, False, c == NCHUNK - 1, c)
                return dict(r0=r0, p_out=p_out)

            def tail(c2):
                r0, p_out = c2["r0"], c2["p_out"]
                a_out = outpool.tile([65, F], dt.float32, tag="aout")
                softplus_rows(p_out, 65, "b_xlo", a_out)
                nc.sync.dma_start(out=out_d.ap()[r0:r0 + F, 0:1], in_=a_out[0:1, :])
                nc.sync.dma_start(out=out_d.ap()[r0 + F:r0 + STB, 0:1], in_=a_out[64:65, :])

            pending = None
            pending2 = None
            for st in range(nst):
                c = body(st)
                if pending is not None:
                    c2 = back(pending)
                    if pending2 is not None:
                        tail(pending2)
                    pending2 = c2
                pending = c
            pending2b = back(pending)
            tail(pending2)
            tail(pending2b)

    nc.finalize()
    return nc


# ---------------------------------------------------------------------------
N_CORES = 8
_program_cache = {}


def _get_program(core_rows):
    if core_rows not in _program_cache:
        _program_cache[core_rows] = build_program(core_rows)
    return _program_cache[core_rows]


def kernel(**inputs):
    from concourse.bass_utils import run_bass_kernel_spmd
    x = np.ascontiguousarray(np.asarray(inputs["input"], dtype=np.float32))
    B = x.shape[0]
    assert x.shape[1] == 2 * D
    core_rows = B // N_CORES
    assert core_rows * N_CORES == B and core_rows % STB == 0, (B,)
    wpack, bpack, ident = pack_weights(inputs)
    nc = _get_program(core_rows)
    in_maps = [{
        "input": x[i * core_rows:(i + 1) * core_rows],
        "wpack": wpack, "bpack": bpack, "ident": ident,
    } for i in range(N_CORES)]
    res = run_bass_kernel_spmd(nc, in_maps, list(range(N_CORES)))
    return np.concatenate([res.results[i]["out"] for i in range(N_CORES)], axis=0)


# revision 7
# speedup vs baseline: 1.1152x; 1.0094x over previous
# Dissipation network Bass kernel for TRN2 (bf16 matmuls, PG-stacked PSUM).
#
# v2: merged per-PG {s|hole|x} matmuls for L2/L3 (one K=50,M=114 instruction
# computes both the s-gate preact and the next x preact per PG, with mirrored
# layouts per PG so DVE gate multiplies see equal operand partition bases),
# plus a combined-K (m1|x0) matmul that folds the W_clin gate matmul into the
# xs1 preact. Softplus = Exp (per-partition bias) then Ln(t + 1), pinned to
# the natural_log_exp_and_others ACT table.
import numpy as np
import ml_dtypes
import concourse.bass as bass
from concourse import bacc
import concourse.hw_specs as hw_specs
import concourse.bacc as bacc_mod
import concourse.mybir as mybir
import concourse.tile as tile

dt = mybir.dt
AF = mybir.ActivationFunctionType
ALU = mybir.AluOpType

_orig_get_tables = hw_specs.get_activation_tables


def _pinned_tables(arch):
    t = _orig_get_tables(arch)
    out = {}
    for name, fns in t.items():
        if name != "natural_log_exp_and_others":
            fns = fns - {AF.Exp, AF.Ln}
        out[name] = fns
    return out


bacc_mod.get_activation_tables = _pinned_tables

D, H = 16, 50
F = 1024            # free columns per PG block
STB = 2 * F         # rows per super-tile
NCHUNK = F // 512   # 512-col matmul chunks per F

W_SPECS = [
    ("W_xin", D, H), ("W_clinm", D, D),
    ("W_xl2", H, H), ("W_cl1m", H, D), ("W_cl1", D, H), ("W_cp1", H, H),
    ("W_xl3", H, H), ("W_cl2m", H, D), ("W_cl2", D, H), ("W_cp2", H, H),
    ("W_xlo", H, 1), ("W_cpom", H, H), ("W_clom", H, D),
    ("W_cpo", H, 1), ("W_clo", D, 1),
]
W_KM = {n: (k, m) for n, k, m in W_SPECS}
X0_WEIGHTS = {"W_xin", "W_clinm"}
W_OFF = {}
_off = 0
for _n, _k, _m in W_SPECS:
    W_OFF[_n] = _off
    _off += _m
W_OFF["W_xs1c"] = _off; _off += H      # combined clin|xl1, K=48
W_OFF["W_m2"] = _off; _off += 114      # merged cp1m|xp1 (mirrored per PG)
W_OFF["W_m3"] = _off; _off += 114      # merged cp2m|xp2
NW = _off

B_SPECS = ["b_xin", "b_clinm", "b_xl1", "b_xl2", "b_cl1m",
           "b_xl3", "b_cl2m", "b_xlo", "b_cpom", "b_clom",
           "b_m2_0", "b_m2_1", "b_m3_0", "b_m3_1"]
B_COL = {n: i for i, n in enumerate(B_SPECS)}
NB = len(B_SPECS)


def pack_weights(inputs):
    def wT(n):
        return np.asarray(inputs[n]).astype(np.float32).T.astype(ml_dtypes.bfloat16)

    wpack = np.zeros((128, NW), dtype=ml_dtypes.bfloat16)
    for n, k, m in W_SPECS:
        wb = wT(n)
        rb = 32 if n in X0_WEIGHTS else 0
        wpack[rb:rb + k, W_OFF[n]:W_OFF[n] + m] = wb
        wpack[64 + rb:64 + rb + k, W_OFF[n]:W_OFF[n] + m] = wb
    c = W_OFF["W_xs1c"]
    wpack[0:16, c:c + H] = wT("W_clin")
    wpack[32:48, c:c + H] = wT("W_xl1")
    wpack[64:80, c:c + H] = wT("W_clin")
    wpack[96:112, c:c + H] = wT("W_xl1")
    # merged {s|hole|x}: PG0 block (rows 0:50) -> out {s@0:50, x@64:114};
    # PG1 block (rows 64:114) -> out {x@0:50, s@64:114}
    for name, ws, wx, flip in (("W_m2", "W_cp1m", "W_xp1", False),
                               ("W_m3", "W_cp2m", "W_xp2", True)):
        c = W_OFF[name]
        sb16, xb16 = wT(ws), wT(wx)
        lo_s, hi_s = (False, True) if flip else (True, False)
        # non-flip: rows 0:50 hold PG0 block {s@0:50, x@64:114};
        # flip (L3): rows 0:50 hold PG1 block {x@0:50, s@64:114}
        if flip:
            wpack[0:50, c:c + 50] = xb16
            wpack[0:50, c + 64:c + 114] = sb16
            wpack[64:114, c:c + 50] = sb16
            wpack[64:114, c + 64:c + 114] = xb16
        else:
            wpack[0:50, c:c + 50] = sb16
            wpack[0:50, c + 64:c + 114] = xb16
            wpack[64:114, c:c + 50] = xb16
            wpack[64:114, c + 64:c + 114] = sb16

    bpack = np.zeros((128, NB), dtype=np.float32)

    def bvec(n):
        return np.asarray(inputs[n]).astype(np.float32)

    for n in ["b_xin", "b_clinm", "b_xl1", "b_xl2", "b_cl1m",
              "b_xl3", "b_cl2m", "b_xlo", "b_cpom", "b_clom"]:
        b = bvec(n)
        cix = B_COL[n]
        bpack[0:len(b), cix] = b
        bpack[64:64 + len(b), cix] = b
    for name, bs, bx in (("b_m2", "b_cp1m", "b_xp1"), ("b_m3", "b_cp2m", "b_xp2")):
        s, x = bvec(bs), bvec(bx)
        bpack[0:50, B_COL[name + "_0"]] = s
        bpack[64:114, B_COL[name + "_0"]] = x
        bpack[0:50, B_COL[name + "_1"]] = x
        bpack[64:114, B_COL[name + "_1"]] = s
    ident = np.eye(128, dtype=np.float32)
    return wpack, bpack, ident


def build_program(n_rows):
    assert n_rows % STB == 0
    nst = n_rows // STB
    nc = bacc.Bacc("TRN2", target_bir_lowering=False, debug=False,
                   enable_asserts=False)
    inp_d = nc.dram_tensor("input", [n_rows, 32], dt.float32, kind="ExternalInput")
    w_d = nc.dram_tensor("wpack", [128, NW], dt.bfloat16, kind="ExternalInput")
    b_d = nc.dram_tensor("bpack", [128, NB], dt.float32, kind="ExternalInput")
    c_d = nc.dram_tensor("ident", [128, 128], dt.float32, kind="ExternalInput")
    out_d = nc.dram_tensor("out", [n_rows, 1], dt.float32, kind="ExternalOutput")

    with tile.TileContext(nc) as tc:
        with tc.tile_pool(name="const", bufs=1) as cpool, \
             tc.tile_pool(name="inp", bufs=4) as inpool, \
             tc.tile_pool(name="x0p", bufs=4) as x0pool, \
             tc.tile_pool(name="mh", bufs=6) as mhpool, \
             tc.tile_pool(name="g", bufs=4) as gpool, \
             tc.tile_pool(name="stg", bufs=8) as stgpool, \
             tc.tile_pool(name="axs", bufs=8) as xspool, \
             tc.tile_pool(name="ax", bufs=4) as xpool, \
             tc.tile_pool(name="uu", bufs=6) as upool, \
             tc.tile_pool(name="aout", bufs=3) as outpool, \
             tc.tile_pool(name="ps", bufs=3, space="PSUM") as ps, \
             tc.tile_pool(name="po", bufs=1, space="PSUM") as po:

            wt = cpool.tile([128, NW], dt.bfloat16)
            nc.sync.dma_start(out=wt[:], in_=w_d.ap())
            bt = cpool.tile([128, NB], dt.float32)
            nc.sync.dma_start(out=bt[:], in_=b_d.ap())
            ct = cpool.tile([128, 128], dt.float32)
            nc.sync.dma_start(out=ct[:], in_=c_d.ap())

            def mm_pg(psum_t, wname, wm, rhs_t, rrow, k, orow,
                      start, stop, c):
                off = W_OFF[wname]
                cs = slice(512 * c, 512 * (c + 1))
                nc.tensor.matmul(psum_t[orow:orow + wm, cs],
                                 wt[rrow:rrow + k, off:off + wm],
                                 rhs_t[rrow:rrow + k, cs],
                                 start=start, stop=stop,
                                 tile_position=(rrow, orow))

            def softplus_rows(psum_t, r1, bias_name, out_t):
                stg = stgpool.tile([r1, F], dt.float32, tag="stg")
                nc.scalar.activation(stg[0:r1, :], psum_t[0:r1, :], AF.Exp,
                                     bias=bt[0:r1, B_COL[bias_name]:B_COL[bias_name] + 1])
                nc.scalar.activation(out_t[0:r1, :], stg[0:r1, :], AF.Ln, bias=1.0)

            def body(st):
                r0 = st * STB
                in_t = inpool.tile([128, 8 * 112], dt.float32, tag="int")
                for pg in range(2):
                    rb = r0 + pg * F
                    src_x = inp_d.ap()[rb:rb + F, 0:16].rearrange("(a p) f -> p a f", p=128)
                    src_s = inp_d.ap()[rb:rb + F, 16:32].rearrange("(a p) f -> p a f", p=128)
                    r3 = in_t[:].rearrange("p (a q) -> p a q", q=112)
                    nc.sync.dma_start(out=r3[:, :, 64 * pg + 32:64 * pg + 48], in_=src_x)
                    nc.sync.dma_start(out=r3[:, :, 64 * pg:64 * pg + 16], in_=src_s)
                    # dup x0s into the 16:32 hole so the combined xs1 matmul
                    # (zero weights there) never multiplies stale SBUF bits
                    nc.sync.dma_start(out=r3[:, :, 64 * pg + 16:64 * pg + 32], in_=src_s)
                pT = ps.tile([112, F], dt.float32, tag="ps")
                for a in range(8):
                    nc.tensor.transpose(pT[0:112, 128 * a:128 * (a + 1)],
                                        in_t[:, 112 * a:112 * a + 112], ct[:])
                x0b = x0pool.tile([112, F], dt.bfloat16, tag="x0b")
                nc.vector.tensor_copy(x0b[0:112, :], pT[0:112, :])
                x0s = x0pool.tile([80, F], dt.float32, tag="x0s")
                nc.vector.tensor_copy(x0s[0:80, :], pT[0:80, :])

                # ---- L1 ----
                p_x1 = ps.tile([114, F], dt.float32, tag="ps")
                for c in range(NCHUNK):
                    mm_pg(p_x1, "W_xin", 50, x0b, 32, 16, 0, True, True, c)
                for c in range(NCHUNK):
                    mm_pg(p_x1, "W_xin", 50, x0b, 96, 16, 64, True, True, c)
                a_x1 = xpool.tile([114, F], dt.bfloat16, tag="ax")
                softplus_rows(p_x1, 114, "b_xin", a_x1)

                p_dm = ps.tile([80, F], dt.float32, tag="ps")
                for c in range(NCHUNK):
                    mm_pg(p_dm, "W_clinm", 16, x0b, 32, 16, 0, True, True, c)
                for c in range(NCHUNK):
                    mm_pg(p_dm, "W_clinm", 16, x0b, 96, 16, 64, True, True, c)
                # m1 -> x0b rows 0:16 / 64:80 (bf16), per PG
                for base in (0, 64):
                    nc.vector.scalar_tensor_tensor(
                        x0b[base:base + 16, :], p_dm[base:base + 16, :],
                        bt[base:base + 16, B_COL["b_clinm"]:B_COL["b_clinm"] + 1],
                        x0s[base:base + 16, :], op0=ALU.add, op1=ALU.mult)
                # xs1 = xl1*x0 + clin*m1 via combined K=48 rhs x0b[0:48]/[64:112]
                p_xs1 = ps.tile([114, F], dt.float32, tag="ps")
                for c in range(NCHUNK):
                    mm_pg(p_xs1, "W_xs1c", 50, x0b, 0, 48, 0, True, True, c)
                for c in range(NCHUNK):
                    mm_pg(p_xs1, "W_xs1c", 50, x0b, 64, 48, 64, True, True, c)
                a_xs1 = xspool.tile([114, F], dt.bfloat16, tag="axs")
                softplus_rows(p_xs1, 114, "b_xl1", a_xs1)
                return dict(r0=r0, x0s=x0s, a_xs1=a_xs1, a_x1=a_x1)

            def layer(a_prev, pg0_r, pg1_r, a_xs, x0s, wm, bm, wdh, bdh,
                      wcl, wxl, wcp, bxs):
                # a_prev x-slots: PG0 at rows pg0_r:pg0_r+50 (tile t0), PG1 at
                # pg1_r:pg1_r+50 (tile t1). For stacked input t0 is t1.
                t0, t1 = a_prev
                A0 = ps.tile([114, F], dt.float32, tag="ps")
                A1 = ps.tile([114, F], dt.float32, tag="ps")
                p_dh = ps.tile([80, F], dt.float32, tag="ps")
                p_xs = ps.tile([114, F], dt.float32, tag="ps")
                for c in range(NCHUNK):
                    mm_pg(A0, wm, 114, t0, pg0_r, 50, 0, True, True, c)
                for c in range(NCHUNK):
                    mm_pg(A1, wm, 114, t1, pg1_r, 50, 0, True, True, c)
                for c in range(NCHUNK):
                    mm_pg(p_dh, wdh, 16, t0, pg0_r, 50, 0, True, True, c)
                for c in range(NCHUNK):
                    mm_pg(p_dh, wdh, 16, t1, pg1_r, 50, 64, True, True, c)
                for c in range(NCHUNK):
                    mm_pg(p_xs, wxl, 50, t0, pg0_r, 50, 0, True, False, c)
                for c in range(NCHUNK):
                    mm_pg(p_xs, wxl, 50, t1, pg1_r, 50, 64, True, False, c)
                U0 = upool.tile([114, F], dt.bfloat16, tag="uu")
                U1 = upool.tile([114, F], dt.bfloat16, tag="uu")
                softplus_rows(A0, 114, bm + "_0", U0)
                softplus_rows(A1, 114, bm + "_1", U1)
                h = mhpool.tile([80, F], dt.bfloat16, tag="mh")
                nc.vector.scalar_tensor_tensor(
                    h[0:80, :], p_dh[0:80, :],
                    bt[0:80, B_COL[bdh]:B_COL[bdh] + 1],
                    x0s[0:80, :], op0=ALU.add, op1=ALU.mult)
                g = gpool.tile([114, F], dt.bfloat16, tag="g")
                nc.vector.tensor_tensor(g[0:50, :], a_xs[0:50, :], U0[0:50, :],
                                        op=ALU.mult)
                nc.vector.tensor_tensor(g[64:114, :], a_xs[64:114, :],
                                        U1[64:114, :], op=ALU.mult)
                for c in range(NCHUNK):
                    mm_pg(p_xs, wcl, 50, h, 0, 16, 0, False, False, c)
                for c in range(NCHUNK):
                    mm_pg(p_xs, wcl, 50, h, 64, 16, 64, False, False, c)
                for c in range(NCHUNK):
                    mm_pg(p_xs, wcp, 50, g, 0, 50, 0, False, c == NCHUNK - 1, c)
                for c in range(NCHUNK):
                    mm_pg(p_xs, wcp, 50, g, 64, 50, 64, False, c == NCHUNK - 1, c)
                a_xs_n = xspool.tile([114, F], dt.bfloat16, tag="axs")
                softplus_rows(p_xs, 114, bxs, a_xs_n)
                return U0, U1, a_xs_n

            def back(cin):
                r0, x0s, a_xs1, a_x1 = cin["r0"], cin["x0s"], cin["a_xs1"], cin["a_x1"]
                U20, U21, a_xs2 = layer((a_x1, a_x1), 0, 64, a_xs1, x0s,
                                        "W_m2", "b_m2", "W_cl1m", "b_cl1m",
                                        "W_cl1", "W_xl2", "W_cp1", "b_xl2")
                # L3: x2 at U20[64:114] (PG0) and U21[0:50] (PG1)
                U30, U31, a_xs3 = layer((U20, U21), 64, 0, a_xs2, x0s,
                                        "W_m3", "b_m3", "W_cl2m", "b_cl2m",
                                        "W_cl2", "W_xl3", "W_cp2", "b_xl3")
                # ---- L4 ----
                p_s3 = ps.tile([114, F], dt.float32, tag="ps")
                p_dh3 = ps.tile([80, F], dt.float32, tag="ps")
                p_out = po.tile([65, F], dt.float32, tag="po")
                for c in range(NCHUNK):
                    mm_pg(p_s3, "W_cpom", 50, U30, 64, 50, 0, True, True, c)
                for c in range(NCHUNK):
                    mm_pg(p_s3, "W_cpom", 50, U31, 0, 50, 64, True, True, c)
                for c in range(NCHUNK):
                    mm_pg(p_dh3, "W_clom", 16, U30, 64, 50, 0, True, True, c)
                for c in range(NCHUNK):
                    mm_pg(p_dh3, "W_clom", 16, U31, 0, 50, 64, True, True, c)
                for c in range(NCHUNK):
                    mm_pg(p_out, "W_xlo", 1, U30, 64, 50, 0, True, False, c)
                for c in range(NCHUNK):
                    mm_pg(p_out, "W_xlo", 1, U31, 0, 50, 64, True, False, c)
                a_s3 = xspool.tile([114, F], dt.bfloat16, tag="axs")
                softplus_rows(p_s3, 114, "b_cpom", a_s3)
                h3 = mhpool.tile([80, F], dt.bfloat16, tag="mh")
                nc.vector.scalar_tensor_tensor(
                    h3[0:80, :], p_dh3[0:80, :],
                    bt[0:80, B_COL["b_clom"]:B_COL["b_clom"] + 1],
                    x0s[0:80, :], op0=ALU.add, op1=ALU.mult)
                g3 = gpool.tile([114, F], dt.bfloat16, tag="g")
                nc.vector.tensor_tensor(g3[0:114, :], a_xs3[0:114, :],
                                        a_s3[0:114, :], op=ALU.mult)
                for c in range(NCHUNK):
                    mm_pg(p_out, "W_clo", 1, h3, 0, 16, 0, False, False, c)
                for c in range(NCHUNK):
                    mm_pg(p_out, "W_clo", 1, h3, 64, 16, 64, False, False, c)
                for c in range(NCHUNK):
                    mm_pg(p_out, "W_cpo", 1, g3, 0, 50, 0, False, c == NCHUNK - 1, c)
                for c in range(NCHUNK):
                    mm_pg(p_out, "W_cpo", 1, g3, 64, 50, 64, False, c == NCHUNK - 1, c)
                return dict(r0=r0, p_out=p_out)

            def tail(c2):
                r0, p_out = c2["r0"], c2["p_out"]
                a_out = outpool.tile([65, F], dt.float32, tag="aout")
                softplus_rows(p_out, 65, "b_xlo", a_out)
                nc.sync.dma_start(out=out_d.ap()[r0:r0 + F, 0:1], in_=a_out[0:1, :])
                nc.sync.dma_start(out=out_d.ap()[r0 + F:r0 + STB, 0:1], in_=a_out[64:65, :])

            pending = None
            pending2 = None
            for st in range(nst):
                c = body(st)
                if pending is not None:
                    c2 = back(pending)
                    if pending2 is not None:
                        tail(pending2)
                    pending2 = c2
                pending = c
            pending2b = back(pending)
            tail(pending2)
            tail(pending2b)

    nc.finalize()
    return nc


# ---------------------------------------------------------------------------
N_CORES = 8
_program_cache = {}


def _get_program(core_rows):
    if core_rows not in _program_cache:
        _program_cache[core_rows] = build_program(core_rows)
    return _program_cache[core_rows]


def kernel(**inputs):
    from concourse.bass_utils import run_bass_kernel_spmd
    x = np.ascontiguousarray(np.asarray(inputs["input"], dtype=np.float32))
    B = x.shape[0]
    assert x.shape[1] == 2 * D
    core_rows = B // N_CORES
    assert core_rows * N_CORES == B and core_rows % STB == 0, (B,)
    wpack, bpack, ident = pack_weights(inputs)
    nc = _get_program(core_rows)
    in_maps = [{
        "input": x[i * core_rows:(i + 1) * core_rows],
        "wpack": wpack, "bpack": bpack, "ident": ident,
    } for i in range(N_CORES)]
    res = run_bass_kernel_spmd(nc, in_maps, list(range(N_CORES)))
    return np.concatenate([res.results[i]["out"] for i in range(N_CORES)], axis=0)
